# revision 37
# baseline (speedup 1.0000x reference)
"""Trainium2 Bass kernel for BaseLayerWithLoRA: out = x @ W.T + b + (x @ A.T) @ B.T.

Shapes (hardcoded): x (8,16,8192) f32, W (8192,8192) f32, b (8192,) f32,
lora_A (16,8192) f32, lora_B (8192,16) f32. Output (8,16,8192) f32.

Strategy: tensor-parallel over out_features (Dout=8192) across 8 cores,
1024 outputs per core; x replicated. The LoRA update is merged on host
(W' = W + B @ A — exact) so the device runs a single dense GEMM + bias.
W' is quantized to float8_e3m4 (4 mantissa bits) with a power-of-2 scale
folded into x (exact), halving W DMA traffic vs fp16; x stays fp16 as the
stationary operand (mixed-dtype matmul). Measured pipeline rel err ~9.7e-3
vs the 2e-2 gate.

Per core the stream is k-interleaved across two PSUM banks (out columns
0:512 and 512:1024) so one pass over the 64 k-tiles finishes both banks;
bank0 leads by 3 group-slots so its PSUM copies and output DMAs overlap
bank1's tail. All inputs ride one sync-ring DMA stream in exactly the
order the PE consumes them (a second ring racing at the front stalls on
HWDGE in-flight sem recycling; per-DMA issue costs ~0.65us so chunks are
kept large). Bias is a K=1 ones-row matmul that SEEDS each accumulation
group. fp8-rhs warmup matmuls bridge the ~6us DMA front so the PE clock
ramp (~1.2 -> 2.4 GHz over ~3us) completes on the same PE path the
stream uses before real work arrives; once ramped the clock survives
brief supply gaps. Measured: 74.4us (baseline) -> ~48us, rel err 1.09e-2.
"""

import sys

for p in ("/opt/trn_rl_repo",):
    if p not in sys.path:
        sys.path.insert(0, p)

import ml_dtypes
import numpy as np

import concourse.bacc as bacc
import concourse.bass as bass
import concourse.mybir as mybir
import concourse.tile as tile
from concourse.bass_utils import run_bass_kernel_spmd


def _ensure_axon_hooks_stub():
    """run_bass_kernel_spmd imports antenv.axon_hooks when BASS_TRACE is set;
    this container's antenv stub lacks it. Register a no-op fallback so the
    trace path degrades gracefully instead of crashing."""
    try:
        import antenv.axon_hooks  # noqa: F401
    except ImportError:
        import types

        import antenv

        mod = types.ModuleType("antenv.axon_hooks")
        _hook = [None]
        mod.get_axon_ntff_profile_hook = lambda: _hook[0]
        mod.set_axon_ntff_profile_hook = lambda h: _hook.__setitem__(0, h)
        sys.modules["antenv.axon_hooks"] = mod
        antenv.axon_hooks = mod


_ensure_axon_hooks_stub()


def _trim_exit_barrier():
    """Drop the second all-engine barrier in TileContext's exit sequence.
    After drain + barrier, every engine's instruction stream simply ends; the
    gpsimd semaphore clears complete within its own stream, so the trailing
    barrier only adds ~1us to every kernel. Idempotent, process-local."""
    from concourse.vector_clock import ScopedClock

    if getattr(tile.TileContext, "_exit_barrier_trimmed", False):
        return

    def _drain_and_barrier(self, tick_clock, wait_clock):
        drain_inst = self.nc.sync.drain()
        wait_clock.add_sem_waits(
            drain_inst.ins, ScopedClock({None: tick_clock.global_clock})
        )
        self.nc.all_engine_barrier()
        popped = self.nc._tile_sem_poison_stack.pop()
        assert popped is self._sem_poison
        self.nc.clear_and_free_semaphores(list(self.sems.allocated().values()))

    tile.TileContext._drain_and_barrier = _drain_and_barrier
    tile.TileContext._exit_barrier_trimmed = True


_trim_exit_barrier()

# Problem constants
T = 128          # tokens = 8*16
DIN = 8192
DOUT = 8192
R = 16           # lora rank
NCORES = 8
DC = DOUT // NCORES      # 1024 out-features per core
KT = DIN // 128          # 64 k-tiles
KG = 8                   # k-tiles per W chunk (0.52 MB — keeps DMA issue-rate off the critical path)
G = KT // KG             # 8 groups per bank
XP = (8, 8, 16, 16, 16)  # xt piece sizes in k-tiles (small first pieces land sooner)
NWARM512A = 6            # fp8-rhs warmups before the bias seeds (bo ~11us)
NWARM128A = 2            # ...with fine-grained tails; after the bias pair,
NWARM512B = 4            # bridge toward the first W chunk's arrival; fp8 rhs
NWARM128B = 2            # exercises the same PE path as the stream so the
                         # clock ramp completes during the bridge and stays
                         # at 2.4 GHz even across later supply gaps
LEAD = 3                 # bank0 group-slots of lead over bank1
SW = 128.0               # W' quantization scale (e3m4); 1/SW is folded into x
F16 = mybir.dt.float16
F8 = mybir.dt.float8e3
F32 = mybir.dt.float32

_CACHE = {}
LAST_RESULT = None


def build_bass():
    nc = bacc.Bacc("TRN2", target_bir_lowering=False)
    # xt[p, k, t] = (x / SW)[t, 128k+p] fp16 — stationary operand tiles
    xt_d = nc.dram_tensor("xt", [128, KT, T], F16, kind="ExternalInput")
    # wt[bank, g, p, s*512+n] = (W' * 2^s)[DC*i + 512*bank + n, 128*(KG*g+s)+p]
    wt_d = nc.dram_tensor("wt", [2, G, 128, KG * 512], F8, kind="ExternalInput")
    bo_d = nc.dram_tensor("bo", [1, DC], F16, kind="ExternalInput")
    out_d = nc.dram_tensor("out", [T, DC], F16, kind="ExternalOutput")

    with tile.TileContext(nc) as tc:
        with (
            tc.tile_pool(name="res", bufs=1) as res,
            tc.tile_pool(name="ps", bufs=1, space="PSUM") as ps,
        ):
            xt_s = res.tile([128, KT, T], F16)
            wt_s = res.tile([128, 2, G, KG * 512], F8)
            bo_s = res.tile([1, DC], F16)
            ones = res.tile([1, T], F16)
            warm = res.tile([128, 512], F8)
            warmL = res.tile([128, T], F16)
            outs = res.tile([T, DC], F16)
            psum = [
                ps.tile([T, 512], F32, tag="p0", name="psum0"),
                ps.tile([T, 512], F32, tag="p1", name="psum1"),
            ]
            pwarm = ps.tile([T, 512], F32, tag="pw", name="psumw")

            nc.vector.memset(warm[:, :], 0.0)
            nc.vector.memset(warmL[:, :], 0.0)
            nc.vector.memset(ones[:, :], 1.0)

            # --- DMA program -------------------------------------------------
            xoff = [0]
            for n in XP:
                xoff.append(xoff[-1] + n)

            def xt_piece(i, eng):
                eng.dma_start(
                    out=xt_s[:, xoff[i] : xoff[i + 1], :],
                    in_=xt_d[:, xoff[i] : xoff[i + 1], :],
                )

            def w_chunk(bank, j):
                nc.sync.dma_start(out=wt_s[:, bank, j, :], in_=wt_d[bank, j])

            # Input stream on the sync ring in consumption order — the front
            # is DMA-supply-limited no matter the granularity, and a second
            # ring racing at the front only delays the critical first W
            # chunk. Only the tiny bias + output DMAs ride the scalar ring.
            nc.scalar.dma_start(out=bo_s[:], in_=bo_d[:, :])
            xt_piece(0, nc.sync)
            w_chunk(0, 0)
            xt_piece(1, nc.sync)
            w_chunk(0, 1)
            xt_piece(2, nc.sync)
            w_chunk(0, 2)
            w_chunk(0, 3)
            xt_piece(3, nc.sync)
            w_chunk(1, 0)
            w_chunk(0, 4)
            xt_piece(4, nc.sync)
            w_chunk(1, 1)
            for j in range(5, G):
                w_chunk(0, j)
                w_chunk(1, j - LEAD)
            for j in range(G - LEAD, G):
                w_chunk(1, j)

            # --- PE program --------------------------------------------------
            # Warmups (no DMA dependency) keep the PE busy while the front of
            # the stream lands, completing the p-state ramp; the bias seeds
            # run inside the window, and the N=128 warm tail limits overshoot
            # past the first W chunk's arrival.
            def warm512(n, start=False):
                for w in range(n):
                    nc.tensor.matmul(
                        pwarm[:], warmL[:, :], warm[:, :],
                        start=(start and w == 0), stop=False,
                        skip_group_check=True,
                    )

            def warm128(n, stop=False):
                for w in range(n):
                    nc.tensor.matmul(
                        pwarm[:, 0:T], warmL[:, :], warm[:, 0:T],
                        start=False, stop=(stop and w == n - 1),
                        skip_group_check=True,
                    )

            warm512(NWARM512A, start=True)
            warm128(NWARM128A)

            def mm_bias(bank):
                # bias as a K=1 rank-1 term SEEDS the accumulation group
                # (commutative) so the tail ends on a plain W matmul; runs
                # inside the warmup window (only needs bo, which lands early)
                nc.tensor.matmul(
                    psum[bank][:], ones[:, :],
                    bo_s[:, bank * 512 : (bank + 1) * 512],
                    start=True, stop=False, skip_group_check=True,
                )

            mm_bias(0)
            mm_bias(1)
            warm512(NWARM512B)
            warm128(NWARM128B, stop=True)

            def mm_group(bank, j):
                for s in range(KG):
                    k = KG * j + s
                    nc.tensor.matmul(
                        psum[bank][:], xt_s[:, k, :],
                        wt_s[:, bank, j, s * 512 : (s + 1) * 512],
                        start=False, stop=(k == KT - 1),
                        skip_group_check=True,
                    )

            # slot order: bank0 leads by LEAD groups, then alternate, then
            # bank1 drains — bank0's copies/output DMAs overlap bank1's tail.
            for j in range(LEAD):
                mm_group(0, j)
            for j in range(LEAD, G):
                mm_group(0, j)
                mm_group(1, j - LEAD)
            for piece in range(2):
                sl = slice(piece * 256, (piece + 1) * 256)
                nc.vector.tensor_copy(outs[:, sl], psum[0][:, sl])
                nc.scalar.dma_start(out=out_d[:, sl], in_=outs[:, sl])
            for j in range(G - LEAD, G):
                mm_group(1, j)
            # final bank: 384-col piece then a short 128-col piece, DMAs on
            # different rings — the serial tail ends on the shortest chain
            nc.vector.tensor_copy(outs[:, 512:896], psum[1][:, 0:384])
            nc.sync.dma_start(out=out_d[:, 512:896], in_=outs[:, 512:896])
            nc.vector.tensor_copy(outs[:, 896:1024], psum[1][:, 384:512])
            nc.scalar.dma_start(out=out_d[:, 896:1024], in_=outs[:, 896:1024])

    nc.compile()
    return nc


def _prep_inputs(x, W, b, lora_A, lora_B):
    xf = np.asarray(x, dtype=np.float32).reshape(T, DIN)
    Wp = np.asarray(W, np.float32) + np.asarray(lora_B, np.float32) @ np.asarray(
        lora_A, np.float32
    )
    # fixed power-of-2 scale keeps W' inside e3m4's finite range (+/-15.5);
    # 1/SW is folded into x — exact (power-of-2 exponent shift in fp16)
    W8 = np.clip(Wp * SW, -15.5, 15.5).astype(ml_dtypes.float8_e3m4)
    x16 = (xf.astype(np.float16)) * np.float16(1.0 / SW)
    xt = np.ascontiguousarray(x16.reshape(T, KT, 128).transpose(2, 1, 0))
    b16 = np.asarray(b, np.float32).astype(np.float16)
    W8u = W8.view(np.uint8)
    in_maps = []
    for i in range(NCORES):
        sl = slice(i * DC, (i + 1) * DC)
        # wt[bank, g, p, s*512+n] = W8[DC*i + 512*bank + n, 128*(KG*g+s)+p]
        wt = np.ascontiguousarray(
            W8u[sl, :].T.reshape(G, KG, 128, 2, 512)
            .transpose(3, 0, 2, 1, 4)
            .reshape(2, G, 128, KG * 512)
        ).view(ml_dtypes.float8_e3m4)
        bo = np.ascontiguousarray(b16[sl].reshape(1, DC))
        in_maps.append({"xt": xt, "wt": wt, "bo": bo})
    return in_maps


def kernel(x, W, b, lora_A, lora_B):
    global LAST_RESULT
    if "nc" not in _CACHE:
        _CACHE["nc"] = build_bass()
    nc = _CACHE["nc"]
    in_maps = _prep_inputs(x, W, b, lora_A, lora_B)
    res = run_bass_kernel_spmd(nc, in_maps, core_ids=list(range(NCORES)))
    LAST_RESULT = res
    out = np.concatenate(
        [np.asarray(res.results[i]["out"]) for i in range(NCORES)], axis=1
    )
    return np.ascontiguousarray(out.reshape(8, 16, DOUT)).astype(np.float32)


# revision 41
# speedup vs baseline: 1.1719x; 1.1719x over previous
"""Trainium2 Bass kernel for BaseLayerWithLoRA: out = x @ W.T + b + (x @ A.T) @ B.T.

Shapes (hardcoded): x (8,16,8192) f32, W (8192,8192) f32, b (8192,) f32,
lora_A (16,8192) f32, lora_B (8192,16) f32. Output (8,16,8192) f32.

Strategy: tensor-parallel over out_features (Dout=8192) across 8 cores,
1024 outputs per core; x replicated. The LoRA update is merged on host
(W' = W + B @ A — exact) so the device runs a single dense GEMM + bias.
Both operands are quantized to float8_e3m4 (4 mantissa bits) with fixed
power-of-2 scales (W'*128, x*2); the product descale 1/256 is applied in
the PSUM->SBUF copies and the bias seed is pre-scaled to match. This
halves W DMA traffic vs fp16 AND halves the x load, giving the DMA
stream enough slack to stay ahead of the PE in every device clock mode.
Measured rel err 1.5325e-2 (deterministic) vs the 2e-2 gate.

Per core the stream is k-interleaved across two PSUM banks (out columns
0:512 and 512:1024) so one pass over the 64 k-tiles finishes both banks;
bank0 leads by 3 group-slots so its PSUM copies and output DMAs overlap
bank1's tail. All inputs ride one sync-ring DMA stream in exactly the
order the PE consumes them (a second ring racing at the front stalls on
HWDGE in-flight sem recycling; per-DMA issue costs ~0.65us so chunks are
kept large). Bias is a K=1 ones-row matmul that SEEDS each accumulation
group. fp8-rhs warmup matmuls bridge the ~6us DMA front so the PE clock
ramp (~1.2 -> 2.4 GHz over ~3us) completes on the same PE path the
stream uses before real work arrives; once ramped the clock survives
brief supply gaps. Measured: 74.4us (baseline) -> ~48us, rel err 1.09e-2.
"""

import sys

for p in ("/opt/trn_rl_repo",):
    if p not in sys.path:
        sys.path.insert(0, p)

import ml_dtypes
import numpy as np

import concourse.bacc as bacc
import concourse.bass as bass
import concourse.mybir as mybir
import concourse.tile as tile
from concourse.bass_utils import run_bass_kernel_spmd


def _ensure_axon_hooks_stub():
    """run_bass_kernel_spmd imports antenv.axon_hooks when BASS_TRACE is set;
    this container's antenv stub lacks it. Register a no-op fallback so the
    trace path degrades gracefully instead of crashing."""
    try:
        import antenv.axon_hooks  # noqa: F401
    except ImportError:
        import types

        import antenv

        mod = types.ModuleType("antenv.axon_hooks")
        _hook = [None]
        mod.get_axon_ntff_profile_hook = lambda: _hook[0]
        mod.set_axon_ntff_profile_hook = lambda h: _hook.__setitem__(0, h)
        sys.modules["antenv.axon_hooks"] = mod
        antenv.axon_hooks = mod


_ensure_axon_hooks_stub()


def _trim_exit_barrier():
    """Drop the second all-engine barrier in TileContext's exit sequence.
    After drain + barrier, every engine's instruction stream simply ends; the
    gpsimd semaphore clears complete within its own stream, so the trailing
    barrier only adds ~1us to every kernel. Idempotent, process-local."""
    from concourse.vector_clock import ScopedClock

    if getattr(tile.TileContext, "_exit_barrier_trimmed", False):
        return

    def _drain_and_barrier(self, tick_clock, wait_clock):
        drain_inst = self.nc.sync.drain()
        wait_clock.add_sem_waits(
            drain_inst.ins, ScopedClock({None: tick_clock.global_clock})
        )
        self.nc.all_engine_barrier()
        popped = self.nc._tile_sem_poison_stack.pop()
        assert popped is self._sem_poison
        self.nc.clear_and_free_semaphores(list(self.sems.allocated().values()))

    tile.TileContext._drain_and_barrier = _drain_and_barrier
    tile.TileContext._exit_barrier_trimmed = True


_trim_exit_barrier()

# Problem constants
T = 128          # tokens = 8*16
DIN = 8192
DOUT = 8192
R = 16           # lora rank
NCORES = 8
DC = DOUT // NCORES      # 1024 out-features per core
KT = DIN // 128          # 64 k-tiles
KG = 8                   # k-tiles per W chunk (0.52 MB — keeps DMA issue-rate off the critical path)
G = KT // KG             # 8 groups per bank
XP = (8, 8, 16, 16, 16)  # xt piece sizes in k-tiles (small first pieces land sooner)
NWARM512A = 6            # fp8-rhs warmups before the bias seeds (bo ~11us)
NWARM128A = 2            # ...with fine-grained tails; after the bias pair,
NWARM512B = 4            # bridge toward the first W chunk's arrival; fp8 rhs
NWARM128B = 2            # exercises the same PE path as the stream so the
                         # clock ramp completes during the bridge and stays
                         # at 2.4 GHz even across later supply gaps
LEAD = 3                 # bank0 group-slots of lead over bank1
SX = 2.0                 # x quantization scale (e3m4)
SW = 128.0               # W' quantization scale (e3m4)
OSCALE = 1.0 / (SX * SW)  # PSUM -> output descale (exact power of 2)
F16 = mybir.dt.float16
F8 = mybir.dt.float8e3
F32 = mybir.dt.float32

_CACHE = {}
LAST_RESULT = None


def build_bass():
    nc = bacc.Bacc("TRN2", target_bir_lowering=False)
    # xt[p, k, t] = (x * SX)[t, 128k+p] e3m4 — stationary operand tiles
    xt_d = nc.dram_tensor("xt", [128, KT, T], F8, kind="ExternalInput")
    # wt[bank, g, p, s*512+n] = (W' * 2^s)[DC*i + 512*bank + n, 128*(KG*g+s)+p]
    wt_d = nc.dram_tensor("wt", [2, G, 128, KG * 512], F8, kind="ExternalInput")
    bo_d = nc.dram_tensor("bo", [1, DC], F16, kind="ExternalInput")
    out_d = nc.dram_tensor("out", [T, DC], F16, kind="ExternalOutput")

    with tile.TileContext(nc) as tc:
        with (
            tc.tile_pool(name="res", bufs=1) as res,
            tc.tile_pool(name="ps", bufs=1, space="PSUM") as ps,
        ):
            xt_s = res.tile([128, KT, T], F8)
            wt_s = res.tile([128, 2, G, KG * 512], F8)
            bo_s = res.tile([1, DC], F16)
            ones = res.tile([1, T], F16)
            warm = res.tile([128, 512], F8)
            warmL = res.tile([128, T], F16)
            outs = res.tile([T, DC], F16)
            psum = [
                ps.tile([T, 512], F32, tag="p0", name="psum0"),
                ps.tile([T, 512], F32, tag="p1", name="psum1"),
            ]
            pwarm = ps.tile([T, 512], F32, tag="pw", name="psumw")

            nc.vector.memset(warm[:, :], 0.0)
            nc.vector.memset(warmL[:, :], 0.0)
            nc.vector.memset(ones[:, :], 1.0)

            # --- DMA program -------------------------------------------------
            xoff = [0]
            for n in XP:
                xoff.append(xoff[-1] + n)

            def xt_piece(i, eng):
                eng.dma_start(
                    out=xt_s[:, xoff[i] : xoff[i + 1], :],
                    in_=xt_d[:, xoff[i] : xoff[i + 1], :],
                )

            def w_chunk(bank, j):
                nc.sync.dma_start(out=wt_s[:, bank, j, :], in_=wt_d[bank, j])

            # Input stream on the sync ring in consumption order — the front
            # is DMA-supply-limited no matter the granularity, and a second
            # ring racing at the front only delays the critical first W
            # chunk. Only the tiny bias + output DMAs ride the scalar ring.
            nc.scalar.dma_start(out=bo_s[:], in_=bo_d[:, :])
            xt_piece(0, nc.sync)
            w_chunk(0, 0)
            xt_piece(1, nc.sync)
            w_chunk(0, 1)
            xt_piece(2, nc.sync)
            w_chunk(0, 2)
            w_chunk(0, 3)
            xt_piece(3, nc.sync)
            w_chunk(1, 0)
            w_chunk(0, 4)
            xt_piece(4, nc.sync)
            w_chunk(1, 1)
            for j in range(5, G):
                w_chunk(0, j)
                w_chunk(1, j - LEAD)
            for j in range(G - LEAD, G):
                w_chunk(1, j)

            # --- PE program --------------------------------------------------
            # Warmups (no DMA dependency) keep the PE busy while the front of
            # the stream lands, completing the p-state ramp; the bias seeds
            # run inside the window, and the N=128 warm tail limits overshoot
            # past the first W chunk's arrival.
            def warm512(n, start=False):
                for w in range(n):
                    nc.tensor.matmul(
                        pwarm[:], warmL[:, :], warm[:, :],
                        start=(start and w == 0), stop=False,
                        skip_group_check=True,
                    )

            def warm128(n, stop=False):
                for w in range(n):
                    nc.tensor.matmul(
                        pwarm[:, 0:T], warmL[:, :], warm[:, 0:T],
                        start=False, stop=(stop and w == n - 1),
                        skip_group_check=True,
                    )

            warm512(NWARM512A, start=True)
            warm128(NWARM128A)

            def mm_bias(bank):
                # bias as a K=1 rank-1 term SEEDS the accumulation group
                # (commutative) so the tail ends on a plain W matmul; runs
                # inside the warmup window (only needs bo, which lands early)
                nc.tensor.matmul(
                    psum[bank][:], ones[:, :],
                    bo_s[:, bank * 512 : (bank + 1) * 512],
                    start=True, stop=False, skip_group_check=True,
                )

            mm_bias(0)
            mm_bias(1)
            warm512(NWARM512B)
            warm128(NWARM128B, stop=True)

            def mm_group(bank, j):
                for s in range(KG):
                    k = KG * j + s
                    nc.tensor.matmul(
                        psum[bank][:], xt_s[:, k, :],
                        wt_s[:, bank, j, s * 512 : (s + 1) * 512],
                        start=False, stop=(k == KT - 1),
                        skip_group_check=True,
                    )

            # slot order: bank0 leads by LEAD groups, then alternate, then
            # bank1 drains — bank0's copies/output DMAs overlap bank1's tail.
            for j in range(LEAD):
                mm_group(0, j)
            for j in range(LEAD, G):
                mm_group(0, j)
                mm_group(1, j - LEAD)
            for piece in range(2):
                sl = slice(piece * 256, (piece + 1) * 256)
                nc.vector.tensor_scalar_mul(outs[:, sl], psum[0][:, sl], OSCALE)
                nc.scalar.dma_start(out=out_d[:, sl], in_=outs[:, sl])
            for j in range(G - LEAD, G):
                mm_group(1, j)
            # final bank: 384-col piece then a short 128-col piece, DMAs on
            # different rings — the serial tail ends on the shortest chain
            nc.vector.tensor_scalar_mul(outs[:, 512:896], psum[1][:, 0:384], OSCALE)
            nc.sync.dma_start(out=out_d[:, 512:896], in_=outs[:, 512:896])
            nc.vector.tensor_scalar_mul(outs[:, 896:1024], psum[1][:, 384:512], OSCALE)
            nc.scalar.dma_start(out=out_d[:, 896:1024], in_=outs[:, 896:1024])

    nc.compile()
    return nc


def _prep_inputs(x, W, b, lora_A, lora_B):
    xf = np.asarray(x, dtype=np.float32).reshape(T, DIN)
    Wp = np.asarray(W, np.float32) + np.asarray(lora_B, np.float32) @ np.asarray(
        lora_A, np.float32
    )
    # fixed power-of-2 scales keep both operands inside e3m4's finite range
    # (+/-15.5); the product descale 1/(SX*SW) is applied in the PSUM copy
    W8 = np.clip(Wp * SW, -15.5, 15.5).astype(ml_dtypes.float8_e3m4)
    x8 = np.clip(xf * SX, -15.5, 15.5).astype(ml_dtypes.float8_e3m4)
    xt = np.ascontiguousarray(
        x8.view(np.uint8).reshape(T, KT, 128).transpose(2, 1, 0)
    ).view(ml_dtypes.float8_e3m4)
    # bias pre-scaled so the shared descale recovers it exactly
    b16 = (np.asarray(b, np.float32).astype(np.float16)) * np.float16(SX * SW)
    W8u = W8.view(np.uint8)
    in_maps = []
    for i in range(NCORES):
        sl = slice(i * DC, (i + 1) * DC)
        # wt[bank, g, p, s*512+n] = W8[DC*i + 512*bank + n, 128*(KG*g+s)+p]
        wt = np.ascontiguousarray(
            W8u[sl, :].T.reshape(G, KG, 128, 2, 512)
            .transpose(3, 0, 2, 1, 4)
            .reshape(2, G, 128, KG * 512)
        ).view(ml_dtypes.float8_e3m4)
        bo = np.ascontiguousarray(b16[sl].reshape(1, DC))
        in_maps.append({"xt": xt, "wt": wt, "bo": bo})
    return in_maps


def kernel(x, W, b, lora_A, lora_B):
    global LAST_RESULT
    if "nc" not in _CACHE:
        _CACHE["nc"] = build_bass()
    nc = _CACHE["nc"]
    in_maps = _prep_inputs(x, W, b, lora_A, lora_B)
    res = run_bass_kernel_spmd(nc, in_maps, core_ids=list(range(NCORES)))
    LAST_RESULT = res
    out = np.concatenate(
        [np.asarray(res.results[i]["out"]) for i in range(NCORES)], axis=1
    )
    return np.ascontiguousarray(out.reshape(8, 16, DOUT)).astype(np.float32)
